# revision 1
# baseline (speedup 1.0000x reference)
"""Trainium2 Bass kernel for nn_AttentionBlock (B=4, C=256, H=W=64, R=32).

Computes: q = Wq@skip + bq; k = Wk@gating + bk; v = Wv@skip + bv
          energy = q^T k per sample; attn = softmax(energy, axis=-1)
          out = gamma * (v @ attn^T) + skip

Sharding: 8 shards = (batch b in 0..3, m-half in 0..1). Each core handles
2048 rows (m) of the 4096x4096 attention matrix for one sample.

Per-core algorithm (matmuls in float32r = TF32-like full-rate, fp32 accum):
  - energy^T chunks [n=128, m=512] = k_chunk^T q via 2x row-tiled K=32
    matmuls (k/q live in partition strips 0:32 and 64:96)
  - expT = exp(energy^T)  (no max subtraction; |energy| < ~10 by construction)
  - out_unnorm [m=128, C+1] = sum_n expT_chunk^T @ [vT | ones]  -- the ones
    column yields the softmax denominator for free
  - final [m, c] = (out_unnorm[:, :C] * (gamma / denom)) + skip^T  (one DVE op)
  - host transposes each core's [2048, 256] result back to [C, m] layout.
"""

import numpy as np

import concourse.bass as bass
import concourse.tile as tile
from concourse import mybir
from concourse import bass_utils

B, C, H, W = 4, 256, 64, 64
N = H * W          # 4096 spatial positions
R = C // 8         # 32 reduced dim
MH = N // 2        # 2048 m rows per core
P = 128            # partitions
NCH = N // P       # 32 n-chunks
MB = 512           # m-block (energy matmul moving width)
NMB = MH // MB     # 4 m-blocks per core
S = MB // P        # 4 m-subtiles per block
CE = C + 2         # extended cols: [v^T | ones | pad] (fp32r needs even N)

F32 = mybir.dt.float32
F32R = mybir.dt.float32r
BF16 = mybir.dt.bfloat16
AF = mybir.ActivationFunctionType
ALU = mybir.AluOpType

_WAIT_LIMIT = 1  # this walrus build supports 1 sync wait per instruction


def _r(ap):
    """View an fp32 AP as float32r (TF32-like matmul mode, same bits)."""
    return ap.bitcast(F32R)


def _split_multi_waits(nc):
    """Hoist excess per-instruction sem waits onto preceding same-engine NOPs.

    The installed walrus rejects >1 sync wait per instruction
    ("Too many sync wait commands"), while Tile freely emits several.
    """
    n_new = 0
    for f in nc.m.functions:
        for blk in f.blocks:
            insts = blk.instructions  # live list reference
            i = 0
            while i < len(insts):
                inst = insts[i]
                si = inst.sync_info
                if si is not None and len(si.on_wait) > _WAIT_LIMIT:
                    waits = list(si.on_wait)
                    si.on_wait = waits[-_WAIT_LIMIT:]
                    for j, w in enumerate(waits[:-_WAIT_LIMIT]):
                        nop = mybir.InstNoOp(
                            name=f"{inst.name}-sw{j}",
                            sync_info=mybir.SyncInfo(on_wait=[w], on_update=[]),
                            bass_nofuse=True,
                            engine=inst.engine,
                        )
                        insts.insert(i, nop)
                        i += 1
                        n_new += 1
                i += 1
    return n_new


def build_nc(split_waits=True):
    nc = bass.Bass("TRN2", target_bir_lowering=False, debug=False)

    # Per-core DRAM inputs (host pre-shards / pre-transposes)
    skf_d = nc.dram_tensor("sk_full", [C, N], BF16, kind="ExternalInput")
    skt_d = nc.dram_tensor("sk_t", [MH, C], F32, kind="ExternalInput")
    gt_d = nc.dram_tensor("gt_full", [C, N], BF16, kind="ExternalInput")
    wqk_d = nc.dram_tensor("wqk", [P, 4 * R], BF16, kind="ExternalInput")
    wvT_d = nc.dram_tensor("wvT", [P, 2 * C], BF16, kind="ExternalInput")
    bqk_d = nc.dram_tensor("bqk", [R, 2], F32, kind="ExternalInput")
    aux_d = nc.dram_tensor("aux", [P, CE + P], BF16, kind="ExternalInput")
    out_d = nc.dram_tensor("out_t", [MH, C], F32, kind="ExternalOutput")

    with tile.TileContext(nc) as tc:
        _body(nc, tc, skf_d, skt_d, gt_d, wqk_d, wvT_d, bqk_d,
              aux_d, out_d)

    if split_waits:
        _split_multi_waits(nc)
    return nc


def _body(nc, tc, skf_d, skt_d, gt_d, wqk_d, wvT_d, bqk_d,
          aux_d, out_d):
    from contextlib import ExitStack
    ctx = ExitStack()
    with ctx:
        cpool = ctx.enter_context(tc.tile_pool(name="const", bufs=1))
        bpool = ctx.enter_context(tc.tile_pool(name="big", bufs=1))
        vtpool = ctx.enter_context(tc.tile_pool(name="vt", bufs=1))
        expool = ctx.enter_context(tc.tile_pool(name="exp", bufs=4))
        smpool = ctx.enter_context(tc.tile_pool(name="small", bufs=4))
        sktpool = ctx.enter_context(tc.tile_pool(name="skt", bufs=1))
        outpool = ctx.enter_context(tc.tile_pool(name="outp", bufs=4))

        # ---- constants / weights (merged into 4 DMA triggers) ----
        # wqk cols: [wqT rows 0:128 | wqT rows 128:256 | wkT 0:128 | wkT 128:256]
        wqk = cpool.tile([P, 4 * R], BF16, name="wqk")
        wvs = cpool.tile([P, 2 * C], BF16, name="wvs")
        bqk = cpool.tile([R, 2], F32, name="bqk")
        auxt = cpool.tile([P, CE + P], BF16, name="auxt")
        nc.sync.dma_start(wqk[:], wqk_d.ap())
        nc.sync.dma_start(wvs[:], wvT_d.ap())
        nc.sync.dma_start(bqk[:], bqk_d.ap())
        nc.sync.dma_start(auxt[:], aux_d.ap())
        wq0 = wqk[:, 0 * R:1 * R]
        wq1 = wqk[:, 1 * R:2 * R]
        wk0 = wqk[:, 2 * R:3 * R]
        wk1 = wqk[:, 3 * R:4 * R]
        wv0 = wvs[:, 0:C]
        wv1 = wvs[:, C:2 * C]
        bqs = bqk[:, 0:1]
        bks = bqk[:, 1:2]
        bve = auxt[:, 0:CE]          # [P, CE], rows 1.. are zero
        ones1 = auxt[:, CE:CE + P]   # [P, P], row 0 ones (rest zero)

        # PE warm-up source (memset first in the DVE stream so warm-up
        # matmuls start as early as possible)
        wrm = cpool.tile([P, MB], BF16, name="wrm")
        nc.vector.memset(wrm[:], 0.0)

        # ---- big activations, split into column halves for earlier starts
        # skip and gating arrive n-PERMUTED per core: cols [m-half | rest].
        # Softmax and the v-weighted sum are n-order invariant, so only the
        # per-core permutation consistency between k, vT and energy matters.
        # q then reads sk cols 0:MH directly (no separate sk_q input).
        sk0 = bpool.tile([P, N], BF16, name="sk0")
        sk1 = bpool.tile([P, N], BF16, name="sk1")
        gt0 = bpool.tile([P, N], BF16, name="gt0")
        gt1 = bpool.tile([P, N], BF16, name="gt1")
        NH = N // 2
        for h in range(2):
            nc.sync.dma_start(sk0[:, h * NH:(h + 1) * NH],
                              skf_d.ap()[0:P, h * NH:(h + 1) * NH])
            nc.sync.dma_start(sk1[:, h * NH:(h + 1) * NH],
                              skf_d.ap()[P:C, h * NH:(h + 1) * NH])
            nc.sync.dma_start(gt0[:, h * NH:(h + 1) * NH],
                              gt_d.ap()[0:P, h * NH:(h + 1) * NH])
            nc.sync.dma_start(gt1[:, h * NH:(h + 1) * NH],
                              gt_d.ap()[P:C, h * NH:(h + 1) * NH])

        # q/k padded to 128 partitions: rows R..128 stay zero so the energy
        # matmuls run with K=128 (the PE activity monitor ignores partial-row
        # matmuls and latches the clock gate cold for K=32 streams).
        # q/k padded to 128 partitions: rows R..128 stay zero so the energy
        # matmuls run with K=128 (the PE activity monitor ignores partial-row
        # matmuls and latches the clock gate cold on K=32 streams).
        # q at partition strips 0:32 and 64:96 (replicated); k chunks
        # 0-15 at strip 0:32, chunks 16-31 at strip 64:96 (cols = n
        # within the half). The row-tiled energy matmuls read only these
        # strips, so no zero padding is needed.
        qsb = bpool.tile([P, MH], BF16, name="qsb")
        ksb = bpool.tile([P, MH], BF16, name="ksb")

        with tc.tile_pool(name="p0psum", bufs=2, space="PSUM") as p0psum:
            pwm = p0psum.tile([P, MB], F32, name="pwm", tag="pwm", bufs=1)
            for wi in range(28):
                nc.tensor.matmul(pwm[:], wrm[:, 0:P], wrm[:],
                                 start=True, stop=True, skip_group_check=True)

            # q = WqT^T @ skip_q + bq -> [R, MH]; 4 m-blocks col-packed
            psq = p0psum.tile([P, MB], F32, name="psq", tag="psq")
            for cc, (wq_c, skq_c) in enumerate(((wq0, sk0), (wq1, sk1))):
                for jb in range(NMB):
                    nc.tensor.matmul(psq[32 * jb:32 * jb + 32, :], wq_c,
                                     skq_c[:, jb * MB:(jb + 1) * MB],
                                     start=(cc == 0), stop=(cc == 1),
                                     tile_position=(0, 32 * jb),
                                     skip_group_check=True)
            for jb in range(NMB):
                nc.scalar.add(qsb[0:R, jb * MB:(jb + 1) * MB],
                              psq[32 * jb:32 * jb + 32, :], bqs[:])
                nc.scalar.add(qsb[64:64 + R, jb * MB:(jb + 1) * MB],
                              psq[32 * jb:32 * jb + 32, :], bqs[:])

            # vT_ext chunks and k, interleaved by DMA arrival order:
            # vT half 0 (sk h0), k half 0 (gt h0), vT half 1, k half 1.
            vts = [None] * NCH

            def emit_vt(j):
                psv = p0psum.tile([P, CE], F32, name=f"psv{j}", tag="psv")
                nc.tensor.matmul(psv[:], ones1[:], bve[:],
                                 start=True, stop=False, skip_group_check=True)
                nc.tensor.matmul(psv[:, 0:C], sk0[:, j * P:(j + 1) * P],
                                 wv0[:],
                                 start=False, stop=False, skip_group_check=True)
                nc.tensor.matmul(psv[:, 0:C], sk1[:, j * P:(j + 1) * P],
                                 wv1[:],
                                 start=False, stop=True, skip_group_check=True)
                vt = vtpool.tile([P, CE], BF16, name=f"vt{j}", tag=f"vt{j}")
                nc.vector.tensor_copy(vt[:], psv[:])
                vts[j] = vt

            def emit_k(half):
                psk = p0psum.tile([P, MB], F32, name=f"psk{half}", tag="psq")
                for cc, (wk_c, gt_c) in enumerate(((wk0, gt0), (wk1, gt1))):
                    for jb in range(4):
                        nb = half * 4 + jb
                        nc.tensor.matmul(psk[32 * jb:32 * jb + 32, :], wk_c,
                                         gt_c[:, nb * MB:(nb + 1) * MB],
                                         start=(cc == 0), stop=(cc == 1),
                                         tile_position=(0, 32 * jb),
                                         skip_group_check=True)
                for jb in range(4):
                    nc.scalar.add(ksb[64 * half:64 * half + R,
                                      jb * MB:(jb + 1) * MB],
                                  psk[32 * jb:32 * jb + 32, :], bks[:])

            for j in range(NCH // 2):
                emit_vt(j)
            emit_k(0)
            for j in range(NCH // 2, NCH):
                emit_vt(j)
            emit_k(1)

        # skip^T for the final residual add: one tile, one DMA
        # skt_all[p, t*C + c] = sk_t[t*128 + p, c]
        skt_all = sktpool.tile([P, NMB * S * C], F32, name="skt_all")
        nc.sync.dma_start(
            skt_all[:].rearrange("p (t c) -> p t c", c=C),
            skt_d.ap().rearrange("(t p) c -> p t c", p=P))
        skts = [skt_all[:, t_i * C:(t_i + 1) * C] for t_i in range(NMB * S)]

        # ---- main attention loop (chunk pairs share one exp) ----
        NPAIR = NCH // 2
        with tc.tile_pool(name="mpsum", bufs=1, space="PSUM") as mpsum:
            for mb in range(NMB):
                mof = mb * MB
                psum_os = [
                    mpsum.tile([P, CE], F32, name=f"po{mb}_{s}", tag="po",
                               bufs=S)
                    for s in range(S)
                ]

                def emit_energy_pair(g, mb=mb, mof=mof):
                    pe2 = mpsum.tile([P, 2 * MB], F32, name=f"pe{mb}_{g}",
                                     tag="pe2", bufs=2)
                    # pair g = chunks (g, g+16): two concurrent K=32
                    # row-tiled matmuls (strips 0:32 / 64:96); the pe2
                    # halves are distinct psum banks (row-tile rule).
                    for jj in range(2):
                        nc.tensor.matmul(pe2[:, jj * MB:(jj + 1) * MB],
                                         ksb[64 * jj:64 * jj + 32,
                                             g * P:(g + 1) * P],
                                         qsb[64 * jj:64 * jj + 32,
                                             mof:mof + MB],
                                         start=True, stop=True,
                                         tile_position=(64 * jj, 0),
                                         skip_group_check=True)
                    ex2 = expool.tile([P, 2 * MB], BF16, name=f"ex{mb}_{g}",
                                      tag="ex")
                    nc.scalar.activation(ex2[:], pe2[:], AF.Exp)
                    return ex2

                def emit_out_pair(g, ex2, psum_os=psum_os, last=False):
                    if last:
                        order = [(s, jj) for s in range(S) for jj in range(2)]
                    else:
                        order = [(s, jj) for jj in range(2) for s in range(S)]
                    for s, jj in order:
                        j = g + 16 * jj
                        nc.tensor.matmul(
                            psum_os[s][:],
                            ex2[:, jj * MB + s * P:jj * MB + (s + 1) * P],
                            vts[j][:],
                            start=(j == 0), stop=(j == NCH - 1),
                            skip_group_check=True)

                # emit energy pairs in bursts of two: within a strip,
                # back-to-back matmuls pipeline (drain overlaps fill), so
                # the following full-array PV matmuls pay the strip-drain
                # latency once per burst instead of once per pair.
                pend = []
                for g2 in range(0, NPAIR, 2):
                    pend.append((g2, emit_energy_pair(g2)))
                    pend.append((g2 + 1, emit_energy_pair(g2 + 1)))
                    while len(pend) > 2:
                        gg, ex = pend.pop(0)
                        emit_out_pair(gg, ex)
                for gg, ex in pend:
                    emit_out_pair(gg, ex, last=(gg == NPAIR - 1))

                for s in range(S):
                    po = psum_os[s]
                    rc = smpool.tile([P, 1], F32, name=f"rc{mb}_{s}", tag="rc")
                    nc.vector.reciprocal(rc[:], po[:, C:C + 1])
                    ob = outpool.tile([P, C], F32, name=f"ob{mb}_{s}", tag="ob")
                    nc.vector.scalar_tensor_tensor(
                        ob[:], po[:, 0:C], rc[:], skts[mb * S + s][:],
                        ALU.mult, ALU.add)
                    row = (mb * S + s) * P
                    nc.sync.dma_start(out_d.ap()[row:row + P, :], ob[:])


_NC_CACHE = None


def _get_nc():
    global _NC_CACHE
    if _NC_CACHE is None:
        _NC_CACHE = build_nc()
    return _NC_CACHE


def make_in_maps(skip, gating, Wq, bq, Wk, bk, Wv, bv, gamma):
    import ml_dtypes
    bf16 = ml_dtypes.bfloat16
    skip = np.ascontiguousarray(np.asarray(skip, np.float32))
    gating = np.ascontiguousarray(np.asarray(gating, np.float32))
    Wq = np.asarray(Wq, np.float32)
    Wk = np.asarray(Wk, np.float32)
    Wv = np.asarray(Wv, np.float32)
    bq = np.asarray(bq, np.float32)
    bk = np.asarray(bk, np.float32)
    bv = np.asarray(bv, np.float32)
    gamma = np.asarray(gamma, np.float32)

    wqT = Wq.T.astype(bf16)
    wkT = Wk.T.astype(bf16)
    g = float(gamma.reshape(-1)[0])
    wvT = (g * Wv).T.astype(bf16)
    wqk = np.ascontiguousarray(np.concatenate(
        [wqT[0:P], wqT[P:C], wkT[0:P], wkT[P:C]], axis=1))
    wvs = np.ascontiguousarray(np.concatenate([wvT[0:P], wvT[P:C]], axis=1))
    bqk = np.ascontiguousarray(
        np.stack([bq, bk], axis=1).astype(np.float32))
    bv_ext = np.concatenate(
        [g * bv, np.ones(1, np.float32), np.zeros(1, np.float32)])
    aux = np.zeros((P, CE + P), np.float32)
    aux[0, 0:CE] = bv_ext
    aux[0, CE:CE + P] = 1.0
    aux = np.ascontiguousarray(aux.astype(bf16))

    in_maps = []
    for s in range(8):
        b, half = divmod(s, 2)
        m0 = half * MH
        skf = skip[b].reshape(C, N)
        gtf = gating[b].reshape(C, N)
        perm = np.r_[m0:m0 + MH, (MH - m0):(N - m0)]  # [m-half | rest]
        skf_b = skf[:, perm].astype(bf16)
        gtf_b = gtf[:, perm].astype(bf16)
        in_maps.append({
            "sk_full": np.ascontiguousarray(skf_b),
            "sk_t": np.ascontiguousarray(skf[:, m0:m0 + MH].T),
            "gt_full": np.ascontiguousarray(gtf_b),
            "wqk": wqk, "wvT": wvs, "bqk": bqk, "aux": aux,
        })
    return in_maps


def gather_outputs(results):
    out = np.empty((B, C, H, W), np.float32)
    outf = out.reshape(B, C, N)
    for s in range(8):
        b, half = divmod(s, 2)
        m0 = half * MH
        outf[b, :, m0:m0 + MH] = results[s]["out_t"].T
    return out


def kernel(skip, gating, Wq, bq, Wk, bk, Wv, bv, gamma, **run_kwargs):
    in_maps = make_in_maps(skip, gating, Wq, bq, Wk, bk, Wv, bv, gamma)
    nc = _get_nc()
    res = bass_utils.run_bass_kernel_spmd(
        nc, in_maps, core_ids=list(range(8)), **run_kwargs)
    out = gather_outputs(res.results)
    if run_kwargs:
        return out, res
    return out

